# revision 5
# baseline (speedup 1.0000x reference)
"""DeformConv1d Trainium2 Bass kernel, v3.

Data-parallel over batch: core b handles batch element b (full CIN x L).

Algorithm per core (vs v2 baseline):
  - conv_off matmuls pack TWO kp taps into the 128-partition contraction
    (stacked rhs tiles: partitions 0:64 = x_g(t), 64:128 = x_g(t+1)),
    cutting conv_off matmul count from 100 to 60 per l-tile.
  - deformable interpolation uses 4 segments (offset clamp [-2,2] instead
    of [-3,3]); piecewise-linear form  val = sum_j d1_j*clamp(off,j,j+1)
    + BASE with BASE = d1(p0-2) + x(p0) - d1(p0+1).
  - softmax reciprocal via Act engine exp(-ln(denom)) instead of DVE
    reciprocal (frees DVE).
  - elementwise work split DVE/GPSIMD by measured cost model (GPSIMD takes
    5 of the wide multiplies).
"""

import os
import numpy as np

B, CIN, COUT, L, K, G = 8, 256, 256, 8192, 5, 4
PAD = 2
MARG = 8
LT = 512
NLT = L // LT
NCH = 2
CPG = CIN // G
NPAIR = 3  # kp pairs (0,1), (2,3), (4,-)

_CACHE = {}
TRACE = os.environ.get("BASS_TRACE", "0") == "1"
LAST_EXEC_NS = None


def _pack_weights(w_off, b_off, weight, bias):
    f16 = np.float16
    w_off_r = w_off.reshape(2, CIN, K, CPG, K)
    # conv group of offset output (d, c, k) is g = 2*d + c//128; its inputs are
    # channels [g*64, g*64+64) = xpad[d] partitions [(c//128)*64, +64).
    # lhsT block for (d, k, ch, pair p): partitions 0:64 = weights for kp=2p,
    # partitions 64:128 = weights for kp=2p+1 (zero when 2p+1 == K).
    wofflhsT = np.zeros((128, 2 * K * NCH * NPAIR, 128), f16)
    for d in range(2):
        for k in range(K):
            for ch in range(NCH):
                for p in range(NPAIR):
                    t = ((d * K + k) * NCH + ch) * NPAIR + p
                    blk = np.zeros((128, 128), np.float32)
                    blk[0:64, :] = w_off_r[d, ch * 128:(ch + 1) * 128, k, :, 2 * p].T
                    if 2 * p + 1 < K:
                        blk[64:128, :] = w_off_r[d, ch * 128:(ch + 1) * 128, k, :, 2 * p + 1].T
                    wofflhsT[:, t, :] = blk.astype(f16)
    w_r = weight.reshape(COUT, CPG, K)
    wfinlhsT = np.zeros((128, K * NCH, 128), f16)
    for k in range(K):
        for ch in range(NCH):
            blk = np.zeros((128, 128), np.float32)
            for half in range(2):
                g = ch * 2 + half
                sub = w_r[g * 64:(g + 1) * 64, :, k]
                blk[half * 64:(half + 1) * 64, half * 64:(half + 1) * 64] = sub.T
            wfinlhsT[:, k * NCH + ch, :] = blk.astype(f16)
    b_off_r = b_off.reshape(2, CIN, K)
    boffs = np.zeros((128, NCH, 2 * K), np.float32)
    for ch in range(NCH):
        for d in range(2):
            for k in range(K):
                boffs[:, ch, d * K + k] = b_off_r[d, ch * 128:(ch + 1) * 128, k]
    bfin = bias.reshape(NCH, 128).T.astype(np.float32).copy()
    p = np.arange(128)
    ones_sm = (p[:, None] % 64 == p[None, :] % 64).astype(f16)
    return (np.ascontiguousarray(wofflhsT), np.ascontiguousarray(wfinlhsT),
            np.ascontiguousarray(boffs), np.ascontiguousarray(bfin),
            np.ascontiguousarray(ones_sm))


def _build(nc):
    import concourse.bass as bass
    import concourse.tile as tile
    import concourse.mybir as mybir
    from concourse.mybir import AluOpType as alu

    def ov(slice_ap, count0, count1):
        """Overlapping [[1,count0],[1,count1]] view anchored at slice_ap's start."""
        return bass.AP(tensor=slice_ap.tensor, offset=slice_ap.offset,
                       ap=[list(slice_ap.ap[0]), [1, count0], [1, count1]])

    f16 = mybir.dt.float16
    f32 = mybir.dt.float32
    AF = mybir.ActivationFunctionType

    x_d = nc.dram_tensor("x", [CIN, L], f32, kind="ExternalInput")
    woff_d = nc.dram_tensor("wofflhsT", [128, 2 * K * NCH * NPAIR, 128], f16, kind="ExternalInput")
    wfin_d = nc.dram_tensor("wfinlhsT", [128, K * NCH, 128], f16, kind="ExternalInput")
    boffs_d = nc.dram_tensor("boffs", [128, NCH, 2 * K], f32, kind="ExternalInput")
    bfin_d = nc.dram_tensor("bfin", [128, NCH], f32, kind="ExternalInput")
    ones_d = nc.dram_tensor("ones_sm", [128, 128], f16, kind="ExternalInput")
    out_d = nc.dram_tensor("out", [COUT, L], f32, kind="ExternalOutput")

    XW = L + 2 * MARG
    PW = LT + 8        # P piece covers x_g over [l0-4, l0+LT+4)
    W1 = LT + 10       # d1A width; d1A[i] = d1(D1LO + i), D1LO = l0 - 6
    WB = LT + 5        # BASE width (i = t + k, t<LT, k<K)

    with tile.TileContext(nc) as tc:
        with (
            tc.tile_pool(name="consts", bufs=1) as consts,
            tc.tile_pool(name="resid", bufs=1) as resid,
            tc.tile_pool(name="stage", bufs=2) as stage,
            tc.tile_pool(name="work", bufs=2) as work,
            tc.tile_pool(name="psc", bufs=4, space="PSUM") as psc,
            tc.tile_pool(name="pss", bufs=2, space="PSUM") as pss,
            tc.tile_pool(name="psf", bufs=2, space="PSUM") as psf,
        ):
            w_sb = consts.tile([128, 2 * K * NCH * NPAIR, 128], f16, name="w_sb", tag="w_sb")
            nc.gpsimd.dma_start(out=w_sb, in_=woff_d[:, :, :])
            wfin_sb = consts.tile([128, K * NCH, 128], f16, name="wfin_sb", tag="wfin_sb")
            nc.gpsimd.dma_start(out=wfin_sb, in_=wfin_d[:, :, :])
            boff_sb = consts.tile([128, NCH, 2 * K], f32, name="boff_sb", tag="boff_sb")
            nc.gpsimd.dma_start(out=boff_sb, in_=boffs_d[:, :, :])
            bfin_sb = consts.tile([128, NCH], f32, name="bfin_sb", tag="bfin_sb")
            nc.gpsimd.dma_start(out=bfin_sb, in_=bfin_d[:, :])
            ones_sb = consts.tile([128, 128], f16, name="ones_sb", tag="ones_sb")
            nc.gpsimd.dma_start(out=ones_sb, in_=ones_d[:, :])

            xpad = []
            for ch in range(NCH):
                xp = resid.tile([128, XW], f16, name=f"xpad{ch}", tag=f"xpad{ch}")
                nc.vector.memset(xp, 0.0)
                xpad.append(xp)
            SST = 1024
            for ch in range(NCH):
                for i in range(L // SST):
                    st = stage.tile([128, SST], f32, name="xstage", tag="xstage")
                    nc.gpsimd.dma_start(
                        out=st, in_=x_d[ch * 128:(ch + 1) * 128, i * SST:(i + 1) * SST])
                    dst = xpad[ch][:, MARG + i * SST:MARG + (i + 1) * SST]
                    if i % 2 == 0:
                        nc.vector.tensor_copy(out=dst, in_=st)
                    else:
                        nc.scalar.activation(out=dst, in_=st, func=AF.Copy,
                                             bias=0.0, scale=1.0)

            for lt in range(NLT):
                l0 = lt * LT

                # stacked rhs pieces for kp-paired conv_off matmuls
                pp = [[None, None], [None, None]]
                for d in range(2):
                    for ch in range(NCH):
                        t = stage.tile([128, PW], f16, name=f"pp{d}{ch}", tag=f"pp{d}{ch}")
                        src0 = MARG + l0 - 4
                        nc.sync.dma_start(
                            out=t[0:64, :],
                            in_=xpad[d][64 * ch:64 * ch + 64, src0:src0 + PW])
                        nc.sync.dma_start(
                            out=t[64:128, :],
                            in_=xpad[d][64 * ch:64 * ch + 64, src0 + 1:src0 + 1 + PW])
                        pp[d][ch] = t

                # conv_off: 3 paired matmuls per (d, k, ch); drain to off/exp f16
                off_t, exp_t = [], []
                for ch in range(NCH):
                    off_t.append(work.tile([128, K, LT], f16, name=f"off_{ch}", tag=f"off_{ch}"))
                    exp_t.append(work.tile([128, K, LT], f16, name=f"exp_{ch}", tag=f"exp_{ch}"))
                for d in range(2):
                    for k in range(K):
                        for ch in range(NCH):
                            ps = psc.tile([128, LT], f32, name="pconv", tag="pconv")
                            for p in range(NPAIR):
                                t = ((d * K + k) * NCH + ch) * NPAIR + p
                                nc.tensor.matmul(
                                    ps,
                                    lhsT=w_sb[:, t, :],
                                    rhs=pp[d][ch][:, 2 * p + 2:2 * p + 2 + LT],
                                    start=(p == 0), stop=(p == NPAIR - 1))
                            if d == 0:
                                nc.scalar.activation(
                                    out=off_t[ch][:, k, :], in_=ps, func=AF.Identity,
                                    bias=boff_sb[:, ch, k:k + 1], scale=1.0)
                            else:
                                nc.scalar.activation(
                                    out=exp_t[ch][:, k, :], in_=ps, func=AF.Exp,
                                    bias=boff_sb[:, ch, K + k:K + k + 1], scale=1.0)

                # softmax denominator -> rec = exp(-ln(denom)) on Act engine
                lg = work.tile([128, K, LT], f16, name="lg", tag="lg", bufs=1)
                for k in range(K):
                    ps = pss.tile([128, LT], f32, name="psm", tag="psm")
                    nc.tensor.matmul(ps, lhsT=ones_sb, rhs=exp_t[0][:, k, :],
                                     start=True, stop=False)
                    nc.tensor.matmul(ps, lhsT=ones_sb, rhs=exp_t[1][:, k, :],
                                     start=False, stop=True)
                    nc.scalar.activation(out=lg[:, k, :], in_=ps, func=AF.Ln,
                                         bias=0.0, scale=1.0)
                rec = work.tile([128, K, LT], f16, name="rec", tag="rec")
                nc.scalar.activation(out=rec, in_=lg, func=AF.Exp, bias=0.0, scale=-1.0)

                # d1 phase copies + BASE per ch
                D1LO = l0 - 6
                d1A, d1B, bs = [], [], []
                for ch in range(NCH):
                    dA = work.tile([128, W1], f16, name=f"d1A_{ch}", tag=f"d1A_{ch}")
                    nc.vector.tensor_tensor(
                        out=dA,
                        in0=xpad[ch][:, MARG + D1LO + 1:MARG + D1LO + 1 + W1],
                        in1=xpad[ch][:, MARG + D1LO:MARG + D1LO + W1],
                        op=alu.subtract)
                    dB = work.tile([128, W1 - 1], f16, name=f"d1B_{ch}", tag=f"d1B_{ch}")
                    nc.gpsimd.tensor_copy(out=dB, in_=dA[:, 1:W1])
                    # BASE(i) = d1(q-2) + x(q) - d1(q+1),  q = l0 - 2 + i
                    bsA = work.tile([128, WB], f16, name=f"bsA_{ch}", tag=f"bsA_{ch}")
                    nc.vector.tensor_tensor(
                        out=bsA, in0=dA[:, 2:2 + WB],
                        in1=xpad[ch][:, MARG + l0 - 2:MARG + l0 - 2 + WB], op=alu.add)
                    b2 = work.tile([128, WB], f16, name=f"bs_{ch}", tag=f"bs_{ch}")
                    nc.vector.tensor_tensor(
                        out=b2, in0=bsA, in1=dB[:, 4:4 + WB], op=alu.subtract)
                    d1A.append(dA)
                    d1B.append(dB)
                    bs.append(b2)

                # deformable interpolation, 4 segments j in {-2,-1,0,1}
                # d1_j view base (k,t): d1 index = t + k + 4 + j
                v_t = []
                for ch in range(NCH):
                    dv = {
                        -2: ov(d1A[ch][:, 2:3], K, LT),
                        -1: ov(d1B[ch][:, 2:3], K, LT),
                        0: ov(d1A[ch][:, 4:5], K, LT),
                        1: ov(d1B[ch][:, 4:5], K, LT),
                    }
                    v = work.tile([128, K, LT], f16, name=f"v_{ch}", tag=f"v_{ch}")
                    u = work.tile([128, K, LT], f16, name=f"u_{ch}", tag=f"u_{ch}", bufs=1)
                    t0 = work.tile([128, K, LT], f16, name=f"t_{ch}", tag=f"t_{ch}")
                    # j = -2 (product on GPSIMD)
                    nc.vector.tensor_scalar(out=u, in0=off_t[ch],
                                            scalar1=-2.0, scalar2=-1.0,
                                            op0=alu.max, op1=alu.min)
                    nc.gpsimd.tensor_tensor(out=v, in0=u, in1=dv[-2], op=alu.mult)
                    # j = -1 (product on GPSIMD)
                    u2 = work.tile([128, K, LT], f16, name=f"u2_{ch}", tag=f"u2_{ch}", bufs=1)
                    nc.vector.tensor_scalar(out=u2, in0=off_t[ch],
                                            scalar1=-1.0, scalar2=0.0,
                                            op0=alu.max, op1=alu.min)
                    nc.gpsimd.tensor_tensor(out=t0, in0=u2, in1=dv[-1], op=alu.mult)
                    nc.vector.tensor_tensor(out=v, in0=v, in1=t0, op=alu.add)
                    # j = 0 (product on GPSIMD for ch 0 only)
                    nc.vector.tensor_scalar(out=u, in0=off_t[ch],
                                            scalar1=0.0, scalar2=1.0,
                                            op0=alu.max, op1=alu.min)
                    eng = nc.gpsimd if ch == 0 else nc.vector
                    eng.tensor_tensor(out=t0, in0=u, in1=dv[0], op=alu.mult)
                    nc.vector.tensor_tensor(out=v, in0=v, in1=t0, op=alu.add)
                    # j = 1
                    nc.vector.tensor_scalar(out=u2, in0=off_t[ch],
                                            scalar1=1.0, scalar2=2.0,
                                            op0=alu.max, op1=alu.min)
                    nc.vector.tensor_tensor(out=t0, in0=u2, in1=dv[1], op=alu.mult)
                    nc.vector.tensor_tensor(out=v, in0=v, in1=t0, op=alu.add)
                    # + BASE, * attn_exp, * rec
                    nc.vector.tensor_tensor(out=v, in0=v, in1=ov(bs[ch][:, 0:1], K, LT),
                                            op=alu.add)
                    nc.vector.tensor_tensor(out=v, in0=v, in1=exp_t[ch], op=alu.mult)
                    nc.vector.tensor_tensor(out=v, in0=v, in1=rec, op=alu.mult)
                    v_t.append(v)

                for ch in range(NCH):
                    ps = psf.tile([128, LT], f32, name="pfin", tag="pfin")
                    for k in range(K):
                        nc.tensor.matmul(
                            ps,
                            lhsT=wfin_sb[:, k * NCH + ch, :],
                            rhs=v_t[ch][:, k, :],
                            start=(k == 0), stop=(k == K - 1))
                    og = work.tile([128, LT], f32, name="og", tag="og")
                    nc.scalar.activation(
                        out=og, in_=ps, func=AF.Identity,
                        bias=bfin_sb[:, ch:ch + 1], scale=1.0)
                    nc.sync.dma_start(
                        out=out_d[ch * 128:(ch + 1) * 128, l0:l0 + LT], in_=og)
    return nc


def _get_compiled():
    if "nc" not in _CACHE:
        import concourse.bacc as bacc
        nc = bacc.Bacc()
        _build(nc)
        nc.compile()
        _CACHE["nc"] = nc
    return _CACHE["nc"]


def kernel(x, w_off, b_off, weight, bias):
    x = np.ascontiguousarray(np.asarray(x, dtype=np.float32))
    w_off = np.asarray(w_off, dtype=np.float32)
    b_off = np.asarray(b_off, dtype=np.float32)
    weight = np.asarray(weight, dtype=np.float32)
    bias = np.asarray(bias, dtype=np.float32)

    wofflhsT, wfinlhsT, boffs, bfin, ones_sm = _pack_weights(w_off, b_off, weight, bias)
    nc = _get_compiled()

    from concourse.bass_utils import run_bass_kernel_spmd
    in_maps = []
    for b in range(B):
        in_maps.append({
            "x": np.ascontiguousarray(x[b]),
            "wofflhsT": wofflhsT,
            "wfinlhsT": wfinlhsT,
            "boffs": boffs,
            "bfin": bfin,
            "ones_sm": ones_sm,
        })
    res = run_bass_kernel_spmd(nc, in_maps, core_ids=list(range(B)),
                               trace=TRACE, stitch_traces=TRACE)
    global LAST_EXEC_NS
    if res.exec_time_ns is not None:
        LAST_EXEC_NS = res.exec_time_ns
    if TRACE and res.instructions_and_trace is not None:
        print("trace:", res.instructions_and_trace[1])
        print("per-core scope times:", res.per_core_scope_times)
    out = np.stack([res.results[b]["out"] for b in range(B)], axis=0)
    return out


# revision 7
# speedup vs baseline: 1.3028x; 1.3028x over previous
"""DeformConv1d Trainium2 Bass kernel, v4.

Data-parallel over batch: core b handles batch element b (full CIN x L).

Structure per core:
  - conv_off (grouped conv -> offsets + attn logits) as kp-PAIRED matmuls:
    two of the five taps share one 128-partition contraction via stacked
    rhs tiles (partitions 0:64 = x_g(t), 64:128 = x_g(t+1)); 60 matmuls
    per 512-wide l-tile instead of 100.
  - deformable sampling as 4-segment piecewise-linear interpolation
    (offset clamped to [-2,2]):
        val = sum_{j=-2..1} d1(p0+j)*clamp(off,j,j+1) + BASE,
        BASE = d1(p0-2) + x(p0) - d1(p0+1)
    with d1 kept in two phase copies (even/odd) so every shifted fp16
    view is 4-byte aligned (keeps DVE 2x packed mode).
  - softmax over the 4 groups: exp at PSUM drain, group-sum via ones
    matmul, reciprocal as exp(-ln(denom)) on the Act engine (the
    activation-table list is pinned at compile so Exp/Ln/Identity/Copy
    all live in one table set -> one table load total).
  - elementwise stage is COLUMN-SPLIT between DVE (t in [0,LA)) and
    GPSIMD/Pool (t in [LA,LT)): each engine runs the whole
    clamp/mult/accumulate chain on its own column range, fully
    decoupled (no cross-engine ping-pong).
"""

import os
import numpy as np

B, CIN, COUT, L, K, G = 8, 256, 256, 8192, 5, 4
PAD = 2
MARG = 8
LT = 512
NLT = L // LT
NCH = 2
CPG = CIN // G
NPAIR = 3   # kp pairs (0,1), (2,3), (4,-)
LA = 404    # DVE handles t in [0,LA), Pool t in [LA,LT); LA must be even

_CACHE = {}
TRACE = os.environ.get("BASS_TRACE", "0") == "1"
LAST_EXEC_NS = None


def _pack_weights(w_off, b_off, weight, bias):
    f16 = np.float16
    w_off_r = w_off.reshape(2, CIN, K, CPG, K)
    # conv group of offset output (d, c, k) is g = 2*d + c//128; its inputs are
    # channels [g*64, g*64+64) = xpad[d] partitions [(c//128)*64, +64).
    wofflhsT = np.zeros((128, 2 * K * NCH * NPAIR, 128), f16)
    for d in range(2):
        for k in range(K):
            for ch in range(NCH):
                for p in range(NPAIR):
                    t = ((d * K + k) * NCH + ch) * NPAIR + p
                    blk = np.zeros((128, 128), np.float32)
                    blk[0:64, :] = w_off_r[d, ch * 128:(ch + 1) * 128, k, :, 2 * p].T
                    if 2 * p + 1 < K:
                        blk[64:128, :] = w_off_r[d, ch * 128:(ch + 1) * 128, k, :, 2 * p + 1].T
                    wofflhsT[:, t, :] = blk.astype(f16)
    w_r = weight.reshape(COUT, CPG, K)
    wfinlhsT = np.zeros((128, K * NCH, 128), f16)
    for k in range(K):
        for ch in range(NCH):
            blk = np.zeros((128, 128), np.float32)
            for half in range(2):
                g = ch * 2 + half
                sub = w_r[g * 64:(g + 1) * 64, :, k]
                blk[half * 64:(half + 1) * 64, half * 64:(half + 1) * 64] = sub.T
            wfinlhsT[:, k * NCH + ch, :] = blk.astype(f16)
    b_off_r = b_off.reshape(2, CIN, K)
    boffs = np.zeros((128, NCH, 2 * K), np.float32)
    for ch in range(NCH):
        for d in range(2):
            for k in range(K):
                boffs[:, ch, d * K + k] = b_off_r[d, ch * 128:(ch + 1) * 128, k]
    bfin = bias.reshape(NCH, 128).T.astype(np.float32).copy()
    p = np.arange(128)
    ones_sm = (p[:, None] % 64 == p[None, :] % 64).astype(f16)
    return (np.ascontiguousarray(wofflhsT), np.ascontiguousarray(wfinlhsT),
            np.ascontiguousarray(boffs), np.ascontiguousarray(bfin),
            np.ascontiguousarray(ones_sm))


def _build(nc):
    import concourse.bass as bass
    import concourse.tile as tile
    import concourse.mybir as mybir
    from concourse.mybir import AluOpType as alu

    def ov(slice_ap, count0, count1):
        """Overlapping [[1,count0],[1,count1]] view anchored at slice_ap's start."""
        return bass.AP(tensor=slice_ap.tensor, offset=slice_ap.offset,
                       ap=[list(slice_ap.ap[0]), [1, count0], [1, count1]])

    f16 = mybir.dt.float16
    f32 = mybir.dt.float32
    AF = mybir.ActivationFunctionType

    x_d = nc.dram_tensor("x", [CIN, L], f32, kind="ExternalInput")
    woff_d = nc.dram_tensor("wofflhsT", [128, 2 * K * NCH * NPAIR, 128], f16, kind="ExternalInput")
    wfin_d = nc.dram_tensor("wfinlhsT", [128, K * NCH, 128], f16, kind="ExternalInput")
    boffs_d = nc.dram_tensor("boffs", [128, NCH, 2 * K], f32, kind="ExternalInput")
    bfin_d = nc.dram_tensor("bfin", [128, NCH], f32, kind="ExternalInput")
    ones_d = nc.dram_tensor("ones_sm", [128, 128], f16, kind="ExternalInput")
    out_d = nc.dram_tensor("out", [COUT, L], f32, kind="ExternalOutput")

    XW = L + 2 * MARG
    PW = LT + 8        # stacked-pair piece covers x_g over [l0-4, l0+LT+4)
    W1 = LT + 10       # d1A[i] = d1(D1LO + i), D1LO = l0 - 6
    WB = LT + 5        # BASE width (i = t + k)
    LB = LT - LA       # Pool column count

    with tile.TileContext(nc) as tc:
        with (
            tc.tile_pool(name="consts", bufs=1) as consts,
            tc.tile_pool(name="resid", bufs=1) as resid,
            tc.tile_pool(name="stage", bufs=2) as stage,
            tc.tile_pool(name="work", bufs=2) as work,
            tc.tile_pool(name="psc", bufs=4, space="PSUM") as psc,
            tc.tile_pool(name="pss", bufs=2, space="PSUM") as pss,
            tc.tile_pool(name="psf", bufs=2, space="PSUM") as psf,
        ):
            w_sb = consts.tile([128, 2 * K * NCH * NPAIR, 128], f16, name="w_sb", tag="w_sb")
            nc.sync.dma_start(out=w_sb, in_=woff_d[:, :, :])
            wfin_sb = consts.tile([128, K * NCH, 128], f16, name="wfin_sb", tag="wfin_sb")
            nc.sync.dma_start(out=wfin_sb, in_=wfin_d[:, :, :])
            boff_sb = consts.tile([128, NCH, 2 * K], f32, name="boff_sb", tag="boff_sb")
            nc.sync.dma_start(out=boff_sb, in_=boffs_d[:, :, :])
            bfin_sb = consts.tile([128, NCH], f32, name="bfin_sb", tag="bfin_sb")
            nc.sync.dma_start(out=bfin_sb, in_=bfin_d[:, :])
            ones_sb = consts.tile([128, 128], f16, name="ones_sb", tag="ones_sb")
            nc.sync.dma_start(out=ones_sb, in_=ones_d[:, :])

            xpad = []
            for ch in range(NCH):
                xp = resid.tile([128, XW], f16, name=f"xpad{ch}", tag=f"xpad{ch}")
                nc.vector.memset(xp[:, 0:MARG], 0.0)
                nc.vector.memset(xp[:, MARG + L:XW], 0.0)
                xpad.append(xp)
            SST = 1024
            for ch in range(NCH):
                for i in range(L // SST):
                    st = stage.tile([128, SST], f32, name="xstage", tag="xstage")
                    nc.sync.dma_start(
                        out=st, in_=x_d[ch * 128:(ch + 1) * 128, i * SST:(i + 1) * SST])
                    dst = xpad[ch][:, MARG + i * SST:MARG + (i + 1) * SST]
                    if i % 2 == 0:
                        nc.vector.tensor_copy(out=dst, in_=st)
                    else:
                        nc.scalar.activation(out=dst, in_=st, func=AF.Copy,
                                             bias=0.0, scale=1.0)

            for lt in range(NLT):
                l0 = lt * LT

                # stacked rhs pieces for kp-paired conv_off matmuls
                pp = [[None, None], [None, None]]
                for d in range(2):
                    for ch in range(NCH):
                        t = stage.tile([128, PW], f16, name=f"pp{d}{ch}", tag=f"pp{d}{ch}")
                        src0 = MARG + l0 - 4
                        nc.sync.dma_start(
                            out=t[0:64, :],
                            in_=xpad[d][64 * ch:64 * ch + 64, src0:src0 + PW])
                        nc.sync.dma_start(
                            out=t[64:128, :],
                            in_=xpad[d][64 * ch:64 * ch + 64, src0 + 1:src0 + 1 + PW])
                        pp[d][ch] = t

                # conv_off: 3 paired matmuls per (d, k, ch); drain to off/exp f16
                off_t, exp_t = [], []
                for ch in range(NCH):
                    off_t.append(work.tile([128, K, LT], f16, name=f"off_{ch}", tag=f"off_{ch}"))
                    exp_t.append(work.tile([128, K, LT], f16, name=f"exp_{ch}", tag=f"exp_{ch}"))
                for d in range(2):
                    for k in range(K):
                        for ch in range(NCH):
                            ps = psc.tile([128, LT], f32, name="pconv", tag="pconv")
                            for p in range(NPAIR):
                                t = ((d * K + k) * NCH + ch) * NPAIR + p
                                nc.tensor.matmul(
                                    ps,
                                    lhsT=w_sb[:, t, :],
                                    rhs=pp[d][ch][:, 2 * p + 2:2 * p + 2 + LT],
                                    start=(p == 0), stop=(p == NPAIR - 1))
                            if d == 0:
                                nc.scalar.activation(
                                    out=off_t[ch][:, k, :], in_=ps, func=AF.Identity,
                                    bias=boff_sb[:, ch, k:k + 1], scale=1.0)
                            else:
                                nc.scalar.activation(
                                    out=exp_t[ch][:, k, :], in_=ps, func=AF.Exp,
                                    bias=boff_sb[:, ch, K + k:K + k + 1], scale=1.0)

                # softmax denominator -> rec = exp(-ln(denom)), all on Act
                rec = work.tile([128, K, LT], f16, name="rec", tag="rec")
                for k in range(K):
                    ps = pss.tile([128, LT], f32, name="psm", tag="psm")
                    nc.tensor.matmul(ps, lhsT=ones_sb, rhs=exp_t[0][:, k, :],
                                     start=True, stop=False)
                    nc.tensor.matmul(ps, lhsT=ones_sb, rhs=exp_t[1][:, k, :],
                                     start=False, stop=True)
                    nc.scalar.activation(out=rec[:, k, :], in_=ps, func=AF.Ln,
                                         bias=0.0, scale=1.0)
                nc.scalar.activation(out=rec, in_=rec, func=AF.Exp, bias=0.0, scale=-1.0)

                # d1 phase copies + BASE per ch (all on DVE; small widths)
                D1LO = l0 - 6
                d1A, d1B, bs = [], [], []
                for ch in range(NCH):
                    dA = work.tile([128, W1], f16, name=f"d1A_{ch}", tag=f"d1A_{ch}")
                    nc.vector.tensor_tensor(
                        out=dA,
                        in0=xpad[ch][:, MARG + D1LO + 1:MARG + D1LO + 1 + W1],
                        in1=xpad[ch][:, MARG + D1LO:MARG + D1LO + W1],
                        op=alu.subtract)
                    dB = work.tile([128, W1 - 1], f16, name=f"d1B_{ch}", tag=f"d1B_{ch}")
                    nc.vector.tensor_copy(out=dB, in_=dA[:, 1:W1])
                    # BASE(i) = d1(q-2) + x(q) - d1(q+1),  q = l0 - 2 + i
                    b2 = work.tile([128, WB], f16, name=f"bs_{ch}", tag=f"bs_{ch}")
                    nc.vector.tensor_tensor(
                        out=b2, in0=dA[:, 2:2 + WB],
                        in1=xpad[ch][:, MARG + l0 - 2:MARG + l0 - 2 + WB], op=alu.add)
                    nc.vector.tensor_tensor(
                        out=b2, in0=b2, in1=dB[:, 4:4 + WB], op=alu.subtract)
                    d1A.append(dA)
                    d1B.append(dB)
                    bs.append(b2)

                # deformable sampling, 4 segments j in {-2,-1,0,1}; columns
                # [0,LA) on DVE, [LA,LT) on Pool -- independent chains.
                # d1_j view base for (k,t): index = t + k + 4 + j into d1A,
                # phase A for even (j=-2: 2+t, j=0: 4+t), B for odd.
                v_t = []
                for ch in range(NCH):
                    v = work.tile([128, K, LT], f16, name=f"v_{ch}", tag=f"v_{ch}")
                    u = work.tile([128, K, LT], f16, name=f"u_{ch}", tag=f"u_{ch}")
                    t0 = work.tile([128, K, LT], f16, name=f"t_{ch}", tag=f"t_{ch}")
                    for eng, c0, cw in ((nc.vector, 0, LA), (nc.gpsimd, LA, LB)):
                        dv = {
                            -2: ov(d1A[ch][:, 2 + c0:3 + c0], K, cw),
                            -1: ov(d1B[ch][:, 2 + c0:3 + c0], K, cw),
                            0: ov(d1A[ch][:, 4 + c0:5 + c0], K, cw),
                            1: ov(d1B[ch][:, 4 + c0:5 + c0], K, cw),
                        }
                        offs = off_t[ch][:, :, c0:c0 + cw]
                        us = u[:, :, c0:c0 + cw]
                        ts = t0[:, :, c0:c0 + cw]
                        vs = v[:, :, c0:c0 + cw]
                        for j in (-2, -1, 0, 1):
                            eng.tensor_scalar(out=us, in0=offs,
                                              scalar1=float(j), scalar2=float(j + 1),
                                              op0=alu.max, op1=alu.min)
                            if j == -2:
                                eng.tensor_tensor(out=vs, in0=us, in1=dv[j], op=alu.mult)
                            else:
                                eng.tensor_tensor(out=ts, in0=us, in1=dv[j], op=alu.mult)
                                eng.tensor_tensor(out=vs, in0=vs, in1=ts, op=alu.add)
                        eng.tensor_tensor(out=vs, in0=vs,
                                          in1=ov(bs[ch][:, c0:c0 + 1], K, cw), op=alu.add)
                        eng.tensor_tensor(out=vs, in0=vs,
                                          in1=exp_t[ch][:, :, c0:c0 + cw], op=alu.mult)
                        eng.tensor_tensor(out=vs, in0=vs,
                                          in1=rec[:, :, c0:c0 + cw], op=alu.mult)
                    v_t.append(v)

                for ch in range(NCH):
                    ps = psf.tile([128, LT], f32, name="pfin", tag="pfin")
                    for k in range(K):
                        nc.tensor.matmul(
                            ps,
                            lhsT=wfin_sb[:, k * NCH + ch, :],
                            rhs=v_t[ch][:, k, :],
                            start=(k == 0), stop=(k == K - 1))
                    og = work.tile([128, LT], f32, name="og", tag="og")
                    nc.scalar.activation(
                        out=og, in_=ps, func=AF.Identity,
                        bias=bfin_sb[:, ch:ch + 1], scale=1.0)
                    nc.sync.dma_start(
                        out=out_d[ch * 128:(ch + 1) * 128, l0:l0 + LT], in_=og)
    return nc


def _get_compiled():
    if "nc" not in _CACHE:
        import concourse.bacc as bacc
        import concourse.hw_specs as hw_specs

        # Pin the activation-table list to the single set that contains every
        # function this kernel uses (exp, ln, identity, copy) so the compile
        # pass emits exactly one table load instead of thrashing between the
        # exp set and the ln set. Restored immediately after compile.
        orig_get_tables = bacc.get_activation_tables

        def pinned_get_tables(arch):
            if os.environ.get("NO_PIN_TABLES", "0") == "1":
                return orig_get_tables(arch)
            tabs = orig_get_tables(arch)
            pinned = {k: v for k, v in tabs.items()
                      if k == "natural_log_exp_and_others"}
            return pinned if pinned else tabs

        nc = bacc.Bacc()
        _build(nc)
        bacc.get_activation_tables = pinned_get_tables
        try:
            nc.compile()
        finally:
            bacc.get_activation_tables = orig_get_tables
        _CACHE["nc"] = nc
    return _CACHE["nc"]


def kernel(x, w_off, b_off, weight, bias):
    x = np.ascontiguousarray(np.asarray(x, dtype=np.float32))
    w_off = np.asarray(w_off, dtype=np.float32)
    b_off = np.asarray(b_off, dtype=np.float32)
    weight = np.asarray(weight, dtype=np.float32)
    bias = np.asarray(bias, dtype=np.float32)

    wofflhsT, wfinlhsT, boffs, bfin, ones_sm = _pack_weights(w_off, b_off, weight, bias)
    nc = _get_compiled()

    from concourse.bass_utils import run_bass_kernel_spmd
    in_maps = []
    for b in range(B):
        in_maps.append({
            "x": np.ascontiguousarray(x[b]),
            "wofflhsT": wofflhsT,
            "wfinlhsT": wfinlhsT,
            "boffs": boffs,
            "bfin": bfin,
            "ones_sm": ones_sm,
        })
    res = run_bass_kernel_spmd(nc, in_maps, core_ids=list(range(B)),
                               trace=TRACE, stitch_traces=TRACE)
    global LAST_EXEC_NS
    if res.exec_time_ns is not None:
        LAST_EXEC_NS = res.exec_time_ns
    if TRACE and res.instructions_and_trace is not None:
        print("trace:", res.instructions_and_trace[1])
        print("per-core scope times:", res.per_core_scope_times)
    out = np.stack([res.results[b]["out"] for b in range(B)], axis=0)
    return out


# revision 12
# speedup vs baseline: 1.3160x; 1.0101x over previous
"""DeformConv1d Trainium2 Bass kernel, v4.

Data-parallel over batch: core b handles batch element b (full CIN x L).

Structure per core:
  - conv_off (grouped conv -> offsets + attn logits) as kp-PAIRED matmuls:
    two of the five taps share one 128-partition contraction via stacked
    rhs tiles (partitions 0:64 = x_g(t), 64:128 = x_g(t+1)); 60 matmuls
    per 512-wide l-tile instead of 100.
  - deformable sampling as 4-segment piecewise-linear interpolation
    (offset clamped to [-2,2]):
        val = sum_{j=-2..1} d1(p0+j)*clamp(off,j,j+1) + BASE,
        BASE = d1(p0-2) + x(p0) - d1(p0+1)
    with d1 kept in two phase copies (even/odd) so every shifted fp16
    view is 4-byte aligned (keeps DVE 2x packed mode).
  - softmax over the 4 groups: exp at PSUM drain, group-sum via ones
    matmul, reciprocal as exp(-ln(denom)) on the Act engine (the
    activation-table list is pinned at compile so Exp/Ln/Identity/Copy
    all live in one table set -> one table load total).
  - elementwise stage is COLUMN-SPLIT between DVE (t in [0,LA)) and
    GPSIMD/Pool (t in [LA,LT)): each engine runs the whole
    clamp/mult/accumulate chain on its own column range, fully
    decoupled (no cross-engine ping-pong).
"""

import os
import numpy as np

B, CIN, COUT, L, K, G = 8, 256, 256, 8192, 5, 4
PAD = 2
MARG = 8
LT = 512
NLT = L // LT
NCH = 2
CPG = CIN // G
NPAIR = 3   # kp pairs (0,1), (2,3), (4,-)
LA = 404    # DVE handles t in [0,LA), Pool t in [LA,LT); LA must be even

_CACHE = {}
TRACE = os.environ.get("BASS_TRACE", "0") == "1"
LAST_EXEC_NS = None


def _pack_weights(w_off, b_off, weight, bias):
    f16 = np.float16
    w_off_r = w_off.reshape(2, CIN, K, CPG, K)
    # conv group of offset output (d, c, k) is g = 2*d + c//128; its inputs are
    # channels [g*64, g*64+64) = xpad[d] partitions [(c//128)*64, +64).
    wofflhsT = np.zeros((128, 2 * K * NCH * NPAIR, 128), f16)
    for d in range(2):
        for k in range(K):
            for ch in range(NCH):
                for p in range(NPAIR):
                    t = ((d * K + k) * NCH + ch) * NPAIR + p
                    blk = np.zeros((128, 128), np.float32)
                    blk[0:64, :] = w_off_r[d, ch * 128:(ch + 1) * 128, k, :, 2 * p].T
                    if 2 * p + 1 < K:
                        blk[64:128, :] = w_off_r[d, ch * 128:(ch + 1) * 128, k, :, 2 * p + 1].T
                    wofflhsT[:, t, :] = blk.astype(f16)
    w_r = weight.reshape(COUT, CPG, K)
    wfinlhsT = np.zeros((128, K * NCH, 128), f16)
    for k in range(K):
        for ch in range(NCH):
            blk = np.zeros((128, 128), np.float32)
            for half in range(2):
                g = ch * 2 + half
                sub = w_r[g * 64:(g + 1) * 64, :, k]
                blk[half * 64:(half + 1) * 64, half * 64:(half + 1) * 64] = sub.T
            wfinlhsT[:, k * NCH + ch, :] = blk.astype(f16)
    b_off_r = b_off.reshape(2, CIN, K)
    boffs = np.zeros((128, NCH, 2 * K), np.float32)
    for ch in range(NCH):
        for d in range(2):
            for k in range(K):
                boffs[:, ch, d * K + k] = b_off_r[d, ch * 128:(ch + 1) * 128, k]
    bfin = bias.reshape(NCH, 128).T.astype(np.float32).copy()
    p = np.arange(128)
    ones_sm = (p[:, None] % 64 == p[None, :] % 64).astype(f16)
    return (np.ascontiguousarray(wofflhsT), np.ascontiguousarray(wfinlhsT),
            np.ascontiguousarray(boffs), np.ascontiguousarray(bfin),
            np.ascontiguousarray(ones_sm))


def _build(nc):
    import concourse.bass as bass
    import concourse.tile as tile
    import concourse.mybir as mybir
    from concourse.mybir import AluOpType as alu

    def ov(slice_ap, count0, count1):
        """Overlapping [[1,count0],[1,count1]] view anchored at slice_ap's start."""
        return bass.AP(tensor=slice_ap.tensor, offset=slice_ap.offset,
                       ap=[list(slice_ap.ap[0]), [1, count0], [1, count1]])

    f16 = mybir.dt.float16
    f32 = mybir.dt.float32
    AF = mybir.ActivationFunctionType

    x_d = nc.dram_tensor("x", [CIN, L], f32, kind="ExternalInput")
    woff_d = nc.dram_tensor("wofflhsT", [128, 2 * K * NCH * NPAIR, 128], f16, kind="ExternalInput")
    wfin_d = nc.dram_tensor("wfinlhsT", [128, K * NCH, 128], f16, kind="ExternalInput")
    boffs_d = nc.dram_tensor("boffs", [128, NCH, 2 * K], f32, kind="ExternalInput")
    bfin_d = nc.dram_tensor("bfin", [128, NCH], f32, kind="ExternalInput")
    ones_d = nc.dram_tensor("ones_sm", [128, 128], f16, kind="ExternalInput")
    out_d = nc.dram_tensor("out", [COUT, L], f32, kind="ExternalOutput")

    XW = L + 2 * MARG
    PW = LT + 8        # stacked-pair piece covers x_g over [l0-4, l0+LT+4)
    W1 = LT + 10       # d1A[i] = d1(D1LO + i), D1LO = l0 - 6
    WB = LT + 5        # BASE width (i = t + k)
    LB = LT - LA       # Pool column count

    with tile.TileContext(nc) as tc:
        with (
            tc.tile_pool(name="consts", bufs=1) as consts,
            tc.tile_pool(name="resid", bufs=1) as resid,
            tc.tile_pool(name="stage", bufs=2) as stage,
            tc.tile_pool(name="work", bufs=2) as work,
            tc.tile_pool(name="psc", bufs=4, space="PSUM") as psc,
            tc.tile_pool(name="pss", bufs=2, space="PSUM") as pss,
            tc.tile_pool(name="psf", bufs=2, space="PSUM") as psf,
        ):
            w_sb = consts.tile([128, 2 * K * NCH * NPAIR, 128], f16, name="w_sb", tag="w_sb")
            nc.sync.dma_start(out=w_sb, in_=woff_d[:, :, :])
            wfin_sb = consts.tile([128, K * NCH, 128], f16, name="wfin_sb", tag="wfin_sb")
            nc.sync.dma_start(out=wfin_sb, in_=wfin_d[:, :, :])
            boff_sb = consts.tile([128, NCH, 2 * K], f32, name="boff_sb", tag="boff_sb")
            nc.sync.dma_start(out=boff_sb, in_=boffs_d[:, :, :])
            bfin_sb = consts.tile([128, NCH], f32, name="bfin_sb", tag="bfin_sb")
            nc.sync.dma_start(out=bfin_sb, in_=bfin_d[:, :])
            ones_sb = consts.tile([128, 128], f16, name="ones_sb", tag="ones_sb")
            nc.sync.dma_start(out=ones_sb, in_=ones_d[:, :])

            xpad = []
            for ch in range(NCH):
                xp = resid.tile([128, XW], f16, name=f"xpad{ch}", tag=f"xpad{ch}")
                nc.vector.memset(xp[:, 0:MARG], 0.0)
                nc.vector.memset(xp[:, MARG + L:XW], 0.0)
                xpad.append(xp)
            SST = 1024
            for ch in range(NCH):
                for i in range(L // SST):
                    st = stage.tile([128, SST], f32, name="xstage", tag="xstage")
                    nc.sync.dma_start(
                        out=st, in_=x_d[ch * 128:(ch + 1) * 128, i * SST:(i + 1) * SST])
                    dst = xpad[ch][:, MARG + i * SST:MARG + (i + 1) * SST]
                    if i % 2 == 0:
                        nc.vector.tensor_copy(out=dst, in_=st)
                    else:
                        nc.scalar.activation(out=dst, in_=st, func=AF.Copy,
                                             bias=0.0, scale=1.0)

            for lt in range(NLT):
                l0 = lt * LT

                # stacked rhs pieces for kp-paired conv_off matmuls
                pp = [[None, None], [None, None]]
                for d in range(2):
                    for ch in range(NCH):
                        t = stage.tile([128, PW], f16, name=f"pp{d}{ch}", tag=f"pp{d}{ch}")
                        src0 = MARG + l0 - 4
                        nc.sync.dma_start(
                            out=t[0:64, :],
                            in_=xpad[d][64 * ch:64 * ch + 64, src0:src0 + PW])
                        nc.sync.dma_start(
                            out=t[64:128, :],
                            in_=xpad[d][64 * ch:64 * ch + 64, src0 + 1:src0 + 1 + PW])
                        pp[d][ch] = t

                # conv_off: 3 paired matmuls per (d, k, ch); drain to off/exp f16
                off_t, exp_t = [], []
                for ch in range(NCH):
                    off_t.append(work.tile([128, K, LT], f16, name=f"off_{ch}", tag=f"off_{ch}"))
                    exp_t.append(work.tile([128, K, LT], f16, name=f"exp_{ch}", tag=f"exp_{ch}"))
                for d in range(2):
                    for k in range(K):
                        for ch in range(NCH):
                            ps = psc.tile([128, LT], f32, name="pconv", tag="pconv")
                            for p in range(NPAIR):
                                t = ((d * K + k) * NCH + ch) * NPAIR + p
                                nc.tensor.matmul(
                                    ps,
                                    lhsT=w_sb[:, t, :],
                                    rhs=pp[d][ch][:, 2 * p + 2:2 * p + 2 + LT],
                                    start=(p == 0), stop=(p == NPAIR - 1))
                            if d == 0:
                                nc.scalar.activation(
                                    out=off_t[ch][:, k, :], in_=ps, func=AF.Identity,
                                    bias=boff_sb[:, ch, k:k + 1], scale=1.0)
                            else:
                                nc.scalar.activation(
                                    out=exp_t[ch][:, k, :], in_=ps, func=AF.Exp,
                                    bias=boff_sb[:, ch, K + k:K + k + 1], scale=1.0)

                # softmax denominator -> rec = exp(-ln(denom)), all on Act
                rec = work.tile([128, K, LT], f16, name="rec", tag="rec")
                for k in range(K):
                    ps = pss.tile([128, LT], f32, name="psm", tag="psm")
                    nc.tensor.matmul(ps, lhsT=ones_sb, rhs=exp_t[0][:, k, :],
                                     start=True, stop=False)
                    nc.tensor.matmul(ps, lhsT=ones_sb, rhs=exp_t[1][:, k, :],
                                     start=False, stop=True)
                    nc.scalar.activation(out=rec[:, k, :], in_=ps, func=AF.Ln,
                                         bias=0.0, scale=1.0)
                nc.scalar.activation(out=rec, in_=rec, func=AF.Exp, bias=0.0, scale=-1.0)

                # d1 phase copies + BASE per ch (all on DVE; small widths)
                D1LO = l0 - 6
                d1A, d1B, bs = [], [], []
                for ch in range(NCH):
                    dA = work.tile([128, W1], f16, name=f"d1A_{ch}", tag=f"d1A_{ch}")
                    nc.vector.tensor_tensor(
                        out=dA,
                        in0=xpad[ch][:, MARG + D1LO + 1:MARG + D1LO + 1 + W1],
                        in1=xpad[ch][:, MARG + D1LO:MARG + D1LO + W1],
                        op=alu.subtract)
                    dB = work.tile([128, W1 - 1], f16, name=f"d1B_{ch}", tag=f"d1B_{ch}")
                    nc.vector.tensor_copy(out=dB, in_=dA[:, 1:W1])
                    # BASE(i) = d1(q-2) + x(q) - d1(q+1),  q = l0 - 2 + i
                    b2 = work.tile([128, WB], f16, name=f"bs_{ch}", tag=f"bs_{ch}")
                    nc.vector.tensor_tensor(
                        out=b2, in0=dA[:, 2:2 + WB],
                        in1=xpad[ch][:, MARG + l0 - 2:MARG + l0 - 2 + WB], op=alu.add)
                    nc.vector.tensor_tensor(
                        out=b2, in0=b2, in1=dB[:, 4:4 + WB], op=alu.subtract)
                    d1A.append(dA)
                    d1B.append(dB)
                    bs.append(b2)

                # deformable sampling, 4 segments j in {-2,-1,0,1}; columns
                # [0,LA) on DVE, [LA,LT) on Pool -- independent chains.
                # d1_j view base for (k,t): index = t + k + 4 + j into d1A,
                # phase A for even j, phase B for odd j.
                v_t = []
                for ch in range(NCH):
                    v = work.tile([128, K, LT], f16, name=f"v_{ch}", tag=f"v_{ch}")
                    u = work.tile([128, K, LT], f16, name=f"u_{ch}", tag=f"u_{ch}")
                    t0 = work.tile([128, K, LT], f16, name=f"t_{ch}", tag=f"t_{ch}")
                    for eng, c0, cw in ((nc.vector, 0, LA), (nc.gpsimd, LA, LT - LA)):
                        dv = {
                            -2: ov(d1A[ch][:, 2 + c0:3 + c0], K, cw),
                            -1: ov(d1B[ch][:, 2 + c0:3 + c0], K, cw),
                            0: ov(d1A[ch][:, 4 + c0:5 + c0], K, cw),
                            1: ov(d1B[ch][:, 4 + c0:5 + c0], K, cw),
                        }
                        offs = off_t[ch][:, :, c0:c0 + cw]
                        us = u[:, :, c0:c0 + cw]
                        ts = t0[:, :, c0:c0 + cw]
                        vs = v[:, :, c0:c0 + cw]
                        for j in (-2, -1, 0, 1):
                            eng.tensor_scalar(out=us, in0=offs,
                                              scalar1=float(j), scalar2=float(j + 1),
                                              op0=alu.max, op1=alu.min)
                            if j == -2:
                                eng.tensor_tensor(out=vs, in0=us, in1=dv[j], op=alu.mult)
                            else:
                                eng.tensor_tensor(out=ts, in0=us, in1=dv[j], op=alu.mult)
                                eng.tensor_tensor(out=vs, in0=vs, in1=ts, op=alu.add)
                        eng.tensor_tensor(out=vs, in0=vs,
                                          in1=ov(bs[ch][:, c0:c0 + 1], K, cw), op=alu.add)
                        eng.tensor_tensor(out=vs, in0=vs,
                                          in1=exp_t[ch][:, :, c0:c0 + cw], op=alu.mult)
                        eng.tensor_tensor(out=vs, in0=vs,
                                          in1=rec[:, :, c0:c0 + cw], op=alu.mult)
                    v_t.append(v)

                for ch in range(NCH):
                    ps = psf.tile([128, LT], f32, name="pfin", tag="pfin")
                    for k in range(K):
                        nc.tensor.matmul(
                            ps,
                            lhsT=wfin_sb[:, k * NCH + ch, :],
                            rhs=v_t[ch][:, k, :],
                            start=(k == 0), stop=(k == K - 1))
                    og = work.tile([128, LT], f32, name="og", tag="og")
                    nc.scalar.activation(
                        out=og, in_=ps, func=AF.Identity,
                        bias=bfin_sb[:, ch:ch + 1], scale=1.0)
                    nc.sync.dma_start(
                        out=out_d[ch * 128:(ch + 1) * 128, l0:l0 + LT], in_=og)
    return nc


def _get_compiled():
    if "nc" not in _CACHE:
        import types
        import concourse.bacc as bacc
        import concourse.mybir as mybir
        import bass_rust

        nc = bacc.Bacc()
        _build(nc)

        if os.environ.get("NO_PIN_TABLES", "0") != "1":
            # Prefer the one activation-table set holding every function this
            # kernel uses (exp, ln, identity, copy) so the load-insertion pass
            # emits a single hoisted table load instead of thrashing between
            # the exp set and the ln set each l-tile. The pass picks the first
            # covering set in list order, and records the set's POSITION in
            # the list it was given as act_func_set_id -- so reorder the list,
            # then rewrite the ids back to true act_info.json indices.
            def pinned_insert_act_table_loads(self):
                tabs = list(bacc.get_activation_tables(self.m.arch).items())
                names = [n for n, _ in tabs]
                tgt = names.index("natural_log_exp_and_others")
                reordered = [tabs[tgt]] + tabs[:tgt] + tabs[tgt + 1:]
                remap = {i: names.index(n) for i, (n, _) in enumerate(reordered)}
                bass_rust.insert_act_table_loads(self, reordered)
                for blk in self.main_func.blocks:
                    for inst in blk.instructions:
                        if isinstance(inst, mybir.InstLoadActFuncSet):
                            inst.act_func_set_id = remap[inst.act_func_set_id]

            nc.insert_act_table_loads = types.MethodType(
                pinned_insert_act_table_loads, nc)

        nc.compile()
        _CACHE["nc"] = nc
    return _CACHE["nc"]


def kernel(x, w_off, b_off, weight, bias):
    x = np.ascontiguousarray(np.asarray(x, dtype=np.float32))
    w_off = np.asarray(w_off, dtype=np.float32)
    b_off = np.asarray(b_off, dtype=np.float32)
    weight = np.asarray(weight, dtype=np.float32)
    bias = np.asarray(bias, dtype=np.float32)

    wofflhsT, wfinlhsT, boffs, bfin, ones_sm = _pack_weights(w_off, b_off, weight, bias)
    nc = _get_compiled()

    from concourse.bass_utils import run_bass_kernel_spmd
    in_maps = []
    for b in range(B):
        in_maps.append({
            "x": np.ascontiguousarray(x[b]),
            "wofflhsT": wofflhsT,
            "wfinlhsT": wfinlhsT,
            "boffs": boffs,
            "bfin": bfin,
            "ones_sm": ones_sm,
        })
    res = run_bass_kernel_spmd(nc, in_maps, core_ids=list(range(B)),
                               trace=TRACE, stitch_traces=TRACE)
    global LAST_EXEC_NS
    if res.exec_time_ns is not None:
        LAST_EXEC_NS = res.exec_time_ns
    if TRACE and res.instructions_and_trace is not None:
        print("trace:", res.instructions_and_trace[1])
        print("per-core scope times:", res.per_core_scope_times)
    out = np.stack([res.results[b]["out"] for b in range(B)], axis=0)
    return out
